# revision 4
# baseline (speedup 1.0000x reference)
"""DynamicScatter3D (segment-max voxelization) on 8 trn2 NeuronCores.

Sharding strategy (host): spatial shard by x>>3 -> core k owns voxels
[k*32768, (k+1)*32768). Within each shard, points are ordered by voxel id
(counting-sort permutation, index-only host prep) and laid out transposed
[channel, position] so the device can run a segmented max-scan along the
free axis. All float arithmetic (the segment max itself) happens on device:

  state = (mask + state) max point     per channel-partition, via
  nc.vector.tensor_tensor_scan(op0=add, op1=max)

with mask = -1e38 at run starts (resets state), 0.0 inside a run. The
run-end position of each voxel then holds the exact fp32 max. Device also
emits count[v] = csr[v+1]-csr[v]. Host extracts run-end columns (pure
indexing) and assembles the dense outputs.
"""

import sys

sys.path.insert(0, "/opt/trn_rl_repo")

import numpy as np
import ml_dtypes

GRID = 64
V = GRID**3  # 262144
C = 64
NCORES = 8
VLOC = V // NCORES  # 32768
F = 4096  # scan tile free size (positions per tile per half)

_cache = {}
run_spmd = None  # test harness may inject a traced runner
LAST_EXEC_NS = None


def _build(np2):
    """Build + compile the SPMD Bass kernel for padded half-length np2."""
    if np2 in _cache:
        return _cache[np2]
    from concourse import bass, bacc, mybir
    import concourse.tile as tile

    nc = bacc.Bacc("TRN2", target_bir_lowering=False, debug=False, num_devices=NCORES)
    pts_in = nc.dram_tensor("ptsT2", [128, np2], mybir.dt.float32, kind="ExternalInput").ap()
    msk_in = nc.dram_tensor("maskT2", [128, np2], mybir.dt.bfloat16, kind="ExternalInput").ap()
    csr_in = nc.dram_tensor("csr", [VLOC + 1], mybir.dt.int32, kind="ExternalInput").ap()
    scan_out = nc.dram_tensor("scan", [128, np2], mybir.dt.float32, kind="ExternalOutput").ap()
    cnt_out = nc.dram_tensor("cnt", [VLOC], mybir.dt.int32, kind="ExternalOutput").ap()

    ntiles = np2 // F
    with tile.TileContext(nc) as tc:
        with (
            tc.tile_pool(name="pts", bufs=3) as ppool,
            tc.tile_pool(name="msk", bufs=3) as mpool,
            tc.tile_pool(name="out", bufs=3) as opool,
            tc.tile_pool(name="misc", bufs=1) as misc,
        ):
            # count = csr[v+1] - csr[v]
            ca = misc.tile([128, VLOC // 128], mybir.dt.int32)
            cb = misc.tile([128, VLOC // 128], mybir.dt.int32)
            nc.gpsimd.dma_start(out=ca[:], in_=csr_in[: VLOC].rearrange("(p r) -> p r", p=128))
            nc.gpsimd.dma_start(out=cb[:], in_=csr_in[1 : VLOC + 1].rearrange("(p r) -> p r", p=128))
            nc.vector.tensor_tensor(out=ca[:], in0=cb[:], in1=ca[:], op=mybir.AluOpType.subtract)
            nc.gpsimd.dma_start(out=cnt_out.rearrange("(p r) -> p r", p=128), in_=ca[:])

            prev = None
            for t in range(ntiles):
                sl = slice(t * F, (t + 1) * F)
                pt = ppool.tile([128, F], mybir.dt.float32)
                nc.sync.dma_start(out=pt[:], in_=pts_in[:, sl])
                mt = mpool.tile([128, F], mybir.dt.float32)
                nc.gpsimd.dma_start(out=mt[:], in_=msk_in[:, sl])  # bf16 -> f32 cast
                ot = opool.tile([128, F], mybir.dt.float32)
                init = -1e38 if prev is None else prev[:, F - 1 : F]
                nc.vector.tensor_tensor_scan(
                    out=ot[:], data0=mt[:], data1=pt[:], initial=init,
                    op0=mybir.AluOpType.add, op1=mybir.AluOpType.max,
                )
                # scalar-engine HWDGE ring: keeps stores off the sync ring
                # that feeds the point loads
                nc.scalar.dma_start(out=scan_out[:, sl], in_=ot[:])
                prev = ot

    nc.compile()
    _cache[np2] = nc
    return nc


def kernel(points: np.ndarray, coors: np.ndarray):
    from concourse.bass_utils import run_bass_kernel_spmd

    N = points.shape[0]
    c = coors.astype(np.int64)
    seg = (c[:, 0] * GRID + c[:, 1]) * GRID + c[:, 2]  # [N] global voxel id
    core = seg // VLOC

    # --- host shard construction (index-only prep) ---
    shards = []
    max_half = 0
    for k in range(NCORES):
        sel = np.flatnonzero(core == k)
        sk = (seg[sel] - k * VLOC).astype(np.int64)
        order = np.argsort(sk, kind="stable")
        psel = sel[order]
        ss = sk[order]
        n = len(ss)
        # CSR pointers for count
        csr = np.zeros(VLOC + 1, np.int32)
        np.cumsum(np.bincount(ss, minlength=VLOC), out=csr[1:])
        # split at a run boundary >= n//2
        starts = np.flatnonzero(np.diff(ss) != 0) + 1  # run-start positions (excl 0)
        half = n // 2
        if n == 0:
            split = 0
        else:
            i = np.searchsorted(starts, half)
            split = int(starts[i]) if i < len(starts) else n
        # run-end positions and their voxels
        ends = np.append(starts, n) - 1 if n else np.array([], np.int64)
        endvox = ss[ends] if n else ends
        shards.append(dict(psel=psel, ss=ss, n=n, split=split, csr=csr,
                           ends=ends, endvox=endvox))
        max_half = max(max_half, split, n - split)

    np2 = max(F, ((max_half + F - 1) // F) * F)
    nc = _build(np2)

    in_maps = []
    for sh in shards:
        n, split = sh["n"], sh["split"]
        ptsT2 = np.zeros((128, np2), np.float32)
        maskb = np.zeros((128, np2), np.float32)
        maskb[:, :] = -1e38  # pad positions: every position its own run
        for h, (a, b) in enumerate(((0, split), (split, n))):
            ln = b - a
            if ln == 0:
                continue
            rows = slice(64 * h, 64 * h + 64)
            ptsT2[rows, :ln] = points[sh["psel"][a:b]].T
            m = np.full(ln, -1e38, np.float32)
            same = np.flatnonzero(np.diff(sh["ss"][a:b]) == 0) + 1
            m[same] = 0.0
            maskb[rows, :ln] = m
        in_maps.append({
            "ptsT2": ptsT2,
            "maskT2": maskb.astype(ml_dtypes.bfloat16),
            "csr": sh["csr"],
        })

    global LAST_EXEC_NS
    runner = run_spmd or run_bass_kernel_spmd
    res = runner(nc, in_maps, list(range(NCORES)))
    LAST_EXEC_NS = res.exec_time_ns

    # --- host unshard: pick run-end columns, assemble dense outputs ---
    voxel_feats = np.zeros((V, C), np.float32)
    count = np.zeros(V, np.int32)
    for k, sh in enumerate(shards):
        r = res.results[k]
        count[k * VLOC : (k + 1) * VLOC] = r["cnt"]
        if sh["n"] == 0:
            continue
        scan = r["scan"]
        ends, split = sh["ends"], sh["split"]
        h = (ends >= split).astype(np.int64)
        i = ends - h * split
        vals = scan[(64 * h)[None, :] + np.arange(64)[:, None], i[None, :]]  # [64, nvox]
        voxel_feats[sh["endvox"] + k * VLOC] = vals.T

    vid = np.arange(V, dtype=np.int32)
    voxel_coors = np.stack([vid // (GRID * GRID), (vid // GRID) % GRID, vid % GRID], axis=1).astype(np.int32)
    return voxel_feats, voxel_coors, count


# revision 7
# speedup vs baseline: 1.4139x; 1.4139x over previous
"""DynamicScatter3D (segment-max voxelization) on 8 trn2 NeuronCores.

Sharding strategy (host): spatial shard by x>>3 -> core k owns voxels
[k*32768, (k+1)*32768). Within each shard, points are ordered by voxel id
(counting-sort permutation, index-only host prep) and laid out transposed
[channel, position] so the device can run a segmented max-scan along the
free axis. All float arithmetic (the segment max itself) happens on device:

  state = (mask + state) max point     per channel-partition, via
  nc.vector.tensor_tensor_scan(op0=add, op1=max)

with mask = -1e38 at run starts (resets state), 0.0 inside a run. The
run-end position of each voxel then holds the exact fp32 max. Device also
emits count[v] = csr[v+1]-csr[v]. Host extracts run-end columns (pure
indexing) and assembles the dense outputs.
"""

import sys

sys.path.insert(0, "/opt/trn_rl_repo")

import numpy as np
import ml_dtypes

GRID = 64
V = GRID**3  # 262144
C = 64
NCORES = 8
VLOC = V // NCORES  # 32768
F = 4096  # scan tile free size (positions per tile per half)

_cache = {}
run_spmd = None  # test harness may inject a traced runner
LAST_EXEC_NS = None


def _build(np2):
    """Build + compile the SPMD Bass kernel for padded half-length np2."""
    if np2 in _cache:
        return _cache[np2]
    from concourse import bass, bacc, mybir
    import concourse.tile as tile

    nc = bacc.Bacc("TRN2", target_bir_lowering=False, debug=False, num_devices=NCORES)
    pts_in = nc.dram_tensor("ptsT2", [128, np2], mybir.dt.float32, kind="ExternalInput").ap()
    msk_in = nc.dram_tensor("maskT2", [128, np2], mybir.dt.uint8, kind="ExternalInput").ap()
    csr_in = nc.dram_tensor("csr", [VLOC + 1], mybir.dt.int32, kind="ExternalInput").ap()
    scan_out = nc.dram_tensor("scan", [128, np2], mybir.dt.float32, kind="ExternalOutput").ap()
    cnt_out = nc.dram_tensor("cnt", [VLOC], mybir.dt.int32, kind="ExternalOutput").ap()

    ntiles = np2 // F
    with tile.TileContext(nc) as tc:
        with (
            tc.tile_pool(name="pts", bufs=3) as ppool,
            tc.tile_pool(name="msk", bufs=3) as mpool,
            tc.tile_pool(name="out", bufs=3) as opool,
            tc.tile_pool(name="misc", bufs=1) as misc,
        ):
            # count = csr[v+1] - csr[v]
            ca = misc.tile([128, VLOC // 128], mybir.dt.int32)
            cb = misc.tile([128, VLOC // 128], mybir.dt.int32)
            nc.gpsimd.dma_start(out=ca[:], in_=csr_in[: VLOC].rearrange("(p r) -> p r", p=128))
            nc.gpsimd.dma_start(out=cb[:], in_=csr_in[1 : VLOC + 1].rearrange("(p r) -> p r", p=128))
            nc.vector.tensor_tensor(out=ca[:], in0=cb[:], in1=ca[:], op=mybir.AluOpType.subtract)
            nc.gpsimd.dma_start(out=cnt_out.rearrange("(p r) -> p r", p=128), in_=ca[:])

            prev = None
            for t in range(ntiles):
                sl = slice(t * F, (t + 1) * F)
                pt = ppool.tile([128, F], mybir.dt.float32)
                nc.sync.dma_start(out=pt[:], in_=pts_in[:, sl])
                mu = mpool.tile([128, F], mybir.dt.uint8, tag="mu")
                nc.sync.dma_start(out=mu[:], in_=msk_in[:, sl])
                mt = mpool.tile([128, F], mybir.dt.float32, tag="mf")
                # u8 {1=boundary} -> f32 {-1e38 boundary, 0 inside run}
                nc.vector.tensor_scalar_mul(mt[:], mu[:], -1e38)
                ot = opool.tile([128, F], mybir.dt.float32)
                init = -1e38 if prev is None else prev[:, F - 1 : F]
                nc.vector.tensor_tensor_scan(
                    out=ot[:], data0=mt[:], data1=pt[:], initial=init,
                    op0=mybir.AluOpType.add, op1=mybir.AluOpType.max,
                )
                # scalar-engine HWDGE ring: keeps stores off the sync ring
                # that feeds the point loads
                nc.scalar.dma_start(out=scan_out[:, sl], in_=ot[:])
                prev = ot

    nc.compile()
    _cache[np2] = nc
    return nc


def kernel(points: np.ndarray, coors: np.ndarray):
    from concourse.bass_utils import run_bass_kernel_spmd

    N = points.shape[0]
    c = coors.astype(np.int64)
    seg = (c[:, 0] * GRID + c[:, 1]) * GRID + c[:, 2]  # [N] global voxel id
    core = seg // VLOC

    # --- host shard construction (index-only prep) ---
    shards = []
    max_half = 0
    for k in range(NCORES):
        sel = np.flatnonzero(core == k)
        sk = (seg[sel] - k * VLOC).astype(np.int64)
        order = np.argsort(sk, kind="stable")
        psel = sel[order]
        ss = sk[order]
        n = len(ss)
        # CSR pointers for count
        csr = np.zeros(VLOC + 1, np.int32)
        np.cumsum(np.bincount(ss, minlength=VLOC), out=csr[1:])
        # split at a run boundary >= n//2
        starts = np.flatnonzero(np.diff(ss) != 0) + 1  # run-start positions (excl 0)
        half = n // 2
        if n == 0:
            split = 0
        else:
            i = np.searchsorted(starts, half)
            split = int(starts[i]) if i < len(starts) else n
        # run-end positions and their voxels
        ends = np.append(starts, n) - 1 if n else np.array([], np.int64)
        endvox = ss[ends] if n else ends
        shards.append(dict(psel=psel, ss=ss, n=n, split=split, csr=csr,
                           ends=ends, endvox=endvox))
        max_half = max(max_half, split, n - split)

    np2 = max(F, ((max_half + F - 1) // F) * F)
    nc = _build(np2)

    in_maps = []
    for sh in shards:
        n, split = sh["n"], sh["split"]
        ptsT2 = np.zeros((128, np2), np.float32)
        maskb = np.ones((128, np2), np.uint8)  # pad: every position its own run
        for h, (a, b) in enumerate(((0, split), (split, n))):
            ln = b - a
            if ln == 0:
                continue
            rows = slice(64 * h, 64 * h + 64)
            ptsT2[rows, :ln] = points[sh["psel"][a:b]].T
            m = np.ones(ln, np.uint8)
            same = np.flatnonzero(np.diff(sh["ss"][a:b]) == 0) + 1
            m[same] = 0
            maskb[rows, :ln] = m
        in_maps.append({
            "ptsT2": ptsT2,
            "maskT2": maskb,
            "csr": sh["csr"],
        })

    global LAST_EXEC_NS
    runner = run_spmd or run_bass_kernel_spmd
    res = runner(nc, in_maps, list(range(NCORES)))
    LAST_EXEC_NS = res.exec_time_ns

    # --- host unshard: pick run-end columns, assemble dense outputs ---
    voxel_feats = np.zeros((V, C), np.float32)
    count = np.zeros(V, np.int32)
    for k, sh in enumerate(shards):
        r = res.results[k]
        count[k * VLOC : (k + 1) * VLOC] = r["cnt"]
        if sh["n"] == 0:
            continue
        scan = r["scan"]
        ends, split = sh["ends"], sh["split"]
        h = (ends >= split).astype(np.int64)
        i = ends - h * split
        vals = scan[(64 * h)[None, :] + np.arange(64)[:, None], i[None, :]]  # [64, nvox]
        voxel_feats[sh["endvox"] + k * VLOC] = vals.T

    vid = np.arange(V, dtype=np.int32)
    voxel_coors = np.stack([vid // (GRID * GRID), (vid // GRID) % GRID, vid % GRID], axis=1).astype(np.int32)
    return voxel_feats, voxel_coors, count


# revision 8
# speedup vs baseline: 1.4401x; 1.0185x over previous
"""DynamicScatter3D (segment-max voxelization) on 8 trn2 NeuronCores.

Sharding strategy (host): spatial shard by x>>3 -> core k owns voxels
[k*32768, (k+1)*32768). Within each shard, points are ordered by voxel id
(counting-sort permutation, index-only host prep) and laid out transposed
[channel, position] so the device can run a segmented max-scan along the
free axis. All float arithmetic (the segment max itself) happens on device:

  state = (mask + state) max point     per channel-partition, via
  nc.vector.tensor_tensor_scan(op0=add, op1=max)

with mask = -1e38 at run starts (resets state), 0.0 inside a run. The
run-end position of each voxel then holds the exact fp32 max. Device also
emits count[v] = csr[v+1]-csr[v]. Host extracts run-end columns (pure
indexing) and assembles the dense outputs.
"""

import sys

sys.path.insert(0, "/opt/trn_rl_repo")

import numpy as np
import ml_dtypes

GRID = 64
V = GRID**3  # 262144
C = 64
NCORES = 8
VLOC = V // NCORES  # 32768
F = 4096  # scan tile free size (positions per tile per half)

_cache = {}
run_spmd = None  # test harness may inject a traced runner
LAST_EXEC_NS = None


def _build(np2):
    """Build + compile the SPMD Bass kernel for padded half-length np2."""
    if np2 in _cache:
        return _cache[np2]
    from concourse import bass, bacc, mybir
    import concourse.tile as tile

    nc = bacc.Bacc("TRN2", target_bir_lowering=False, debug=False, num_devices=NCORES)
    pts_in = nc.dram_tensor("ptsT2", [128, np2], mybir.dt.float32, kind="ExternalInput").ap()
    msk_in = nc.dram_tensor("maskT2", [128, np2], mybir.dt.uint8, kind="ExternalInput").ap()
    csr_in = nc.dram_tensor("csr", [VLOC + 1], mybir.dt.int32, kind="ExternalInput").ap()
    scan_out = nc.dram_tensor("scan", [128, np2], mybir.dt.float32, kind="ExternalOutput").ap()
    cnt_out = nc.dram_tensor("cnt", [VLOC], mybir.dt.int32, kind="ExternalOutput").ap()

    ntiles = np2 // F
    with tile.TileContext(nc) as tc:
        with (
            tc.tile_pool(name="pts", bufs=3) as ppool,
            tc.tile_pool(name="msk", bufs=3) as mpool,
            tc.tile_pool(name="out", bufs=3) as opool,
            tc.tile_pool(name="misc", bufs=1) as misc,
        ):
            # count = csr[v+1] - csr[v]
            ca = misc.tile([128, VLOC // 128], mybir.dt.int32)
            cb = misc.tile([128, VLOC // 128], mybir.dt.int32)
            nc.gpsimd.dma_start(out=ca[:], in_=csr_in[: VLOC].rearrange("(p r) -> p r", p=128))
            nc.gpsimd.dma_start(out=cb[:], in_=csr_in[1 : VLOC + 1].rearrange("(p r) -> p r", p=128))
            nc.vector.tensor_tensor(out=ca[:], in0=cb[:], in1=ca[:], op=mybir.AluOpType.subtract)
            nc.gpsimd.dma_start(out=cnt_out.rearrange("(p r) -> p r", p=128), in_=ca[:])

            prev = None
            for t in range(ntiles):
                sl = slice(t * F, (t + 1) * F)
                pt = ppool.tile([128, F], mybir.dt.float32)
                nc.sync.dma_start(out=pt[:], in_=pts_in[:, sl])
                mu = mpool.tile([128, F], mybir.dt.uint8, tag="mu")
                nc.sync.dma_start(out=mu[:], in_=msk_in[:, sl])
                mt = mpool.tile([128, F], mybir.dt.float32, tag="mf")
                # u8 {1=boundary} -> f32 {-1e38 boundary, 0 inside run}.
                # On ScalarE (idle) so the DVE only runs the scan itself.
                nc.scalar.activation(
                    out=mt[:], in_=mu[:],
                    func=mybir.ActivationFunctionType.Copy, scale=-1e38,
                )
                ot = opool.tile([128, F], mybir.dt.float32)
                init = -1e38 if prev is None else prev[:, F - 1 : F]
                nc.vector.tensor_tensor_scan(
                    out=ot[:], data0=mt[:], data1=pt[:], initial=init,
                    op0=mybir.AluOpType.add, op1=mybir.AluOpType.max,
                )
                # scalar-engine HWDGE ring: keeps stores off the sync ring
                # that feeds the point loads
                nc.scalar.dma_start(out=scan_out[:, sl], in_=ot[:])
                prev = ot

    nc.compile()
    _cache[np2] = nc
    return nc


def kernel(points: np.ndarray, coors: np.ndarray):
    from concourse.bass_utils import run_bass_kernel_spmd

    N = points.shape[0]
    c = coors.astype(np.int64)
    seg = (c[:, 0] * GRID + c[:, 1]) * GRID + c[:, 2]  # [N] global voxel id
    core = seg // VLOC

    # --- host shard construction (index-only prep) ---
    shards = []
    max_half = 0
    for k in range(NCORES):
        sel = np.flatnonzero(core == k)
        sk = (seg[sel] - k * VLOC).astype(np.int64)
        order = np.argsort(sk, kind="stable")
        psel = sel[order]
        ss = sk[order]
        n = len(ss)
        # CSR pointers for count
        csr = np.zeros(VLOC + 1, np.int32)
        np.cumsum(np.bincount(ss, minlength=VLOC), out=csr[1:])
        # split at a run boundary >= n//2
        starts = np.flatnonzero(np.diff(ss) != 0) + 1  # run-start positions (excl 0)
        half = n // 2
        if n == 0:
            split = 0
        else:
            i = np.searchsorted(starts, half)
            split = int(starts[i]) if i < len(starts) else n
        # run-end positions and their voxels
        ends = np.append(starts, n) - 1 if n else np.array([], np.int64)
        endvox = ss[ends] if n else ends
        shards.append(dict(psel=psel, ss=ss, n=n, split=split, csr=csr,
                           ends=ends, endvox=endvox))
        max_half = max(max_half, split, n - split)

    np2 = max(F, ((max_half + F - 1) // F) * F)
    nc = _build(np2)

    in_maps = []
    for sh in shards:
        n, split = sh["n"], sh["split"]
        ptsT2 = np.zeros((128, np2), np.float32)
        maskb = np.ones((128, np2), np.uint8)  # pad: every position its own run
        for h, (a, b) in enumerate(((0, split), (split, n))):
            ln = b - a
            if ln == 0:
                continue
            rows = slice(64 * h, 64 * h + 64)
            ptsT2[rows, :ln] = points[sh["psel"][a:b]].T
            m = np.ones(ln, np.uint8)
            same = np.flatnonzero(np.diff(sh["ss"][a:b]) == 0) + 1
            m[same] = 0
            maskb[rows, :ln] = m
        in_maps.append({
            "ptsT2": ptsT2,
            "maskT2": maskb,
            "csr": sh["csr"],
        })

    global LAST_EXEC_NS
    runner = run_spmd or run_bass_kernel_spmd
    res = runner(nc, in_maps, list(range(NCORES)))
    LAST_EXEC_NS = res.exec_time_ns

    # --- host unshard: pick run-end columns, assemble dense outputs ---
    voxel_feats = np.zeros((V, C), np.float32)
    count = np.zeros(V, np.int32)
    for k, sh in enumerate(shards):
        r = res.results[k]
        count[k * VLOC : (k + 1) * VLOC] = r["cnt"]
        if sh["n"] == 0:
            continue
        scan = r["scan"]
        ends, split = sh["ends"], sh["split"]
        h = (ends >= split).astype(np.int64)
        i = ends - h * split
        vals = scan[(64 * h)[None, :] + np.arange(64)[:, None], i[None, :]]  # [64, nvox]
        voxel_feats[sh["endvox"] + k * VLOC] = vals.T

    vid = np.arange(V, dtype=np.int32)
    voxel_coors = np.stack([vid // (GRID * GRID), (vid // GRID) % GRID, vid % GRID], axis=1).astype(np.int32)
    return voxel_feats, voxel_coors, count
